# revision 1
# baseline (speedup 1.0000x reference)
"""DGCNN (3x DynamicEdgeConv, kNN=20) Trainium2 Bass kernel.

Self-contained: `kernel(**inputs) -> np.ndarray` takes the full inputs from
setup_inputs() (pos [8,4096,3] + 9 weight/bias pairs) and returns [8,4096,64].

Sharding: data-parallel over batch B=8 -> one point cloud per NeuronCore,
weights replicated. Each core runs the identical program on its slice.

Per-core, per-layer pipeline (N=4096 points, feature dim D in {3,64}, H=64):
  phase 0: X2T = 2*X^T in SBUF [64,4096]; negsq2 = -2*||x_j||^2 row;
           U = x@(W1a-W1b)+b1 (point-major, SBUF), V = x@W1b -> DRAM [4096,64]
  per row-tile t (128 points):
    S = 4 x_i.x_j - 2 sq_j  (PE, accumulating -2sq row; row-monotone == -dist)
    top-20: per-128-chunk top-8 via DVE max/max_index -> 256 candidates,
            3x (max8 + match_replace) -> 24 winners, positions via max_index,
            per-row candidate-index lookup via gpsimd indirect_copy + 16
            strided-partition diagonal DMAs -> idx [128,24] (first 20 valid)
    h1 = leaky(U_i + V_j): prefill U (ACT), 20x indirect DMA gather with
            CCE-add, Prelu(alpha=0.2)
    h1 -> 20 PE transposes -> h1T [64, 20*128] (edge k-major)
    h2T = Prelu(W2^T@h1T + b2), h3T = Prelu(W3^T@h2T + b3)   (PE + ACT)
    out tile = max over k (DVE strided reduce) -> next layer's X2T
"""
import numpy as np

import concourse.bass as bass
import concourse.bacc as bacc
import concourse.mybir as mybir
import concourse.tile as tile
from concourse.bass_utils import run_bass_kernel_spmd
from concourse.masks import make_identity

F32 = mybir.dt.float32
U16 = mybir.dt.uint16
U32 = mybir.dt.uint32
I16 = mybir.dt.int16
AF = mybir.ActivationFunctionType
ALU = mybir.AluOpType

B = 8
N = 4096
P = 128
NT = N // P            # 32 row tiles
CS = 128               # top-k chunk size
NCH = N // CS          # 32 chunks
K = 20
H = 64
SLOPE = 0.2
NEG = -3.0e38
import os
SKIP_GATHER = bool(int(os.environ.get("SKIP_GATHER", "0")))
SKIP_TOPK = bool(int(os.environ.get("SKIP_TOPK", "0")))
FP32R = bool(int(os.environ.get("FP32R", "0")))
STAGE = int(os.environ.get("STAGE", "4"))  # truncate per-tile pipeline for attribution
SIM_ACT = bool(int(os.environ.get("SIM_ACT", "0")))  # Prelu->Copy for CoreSim
GMODE = os.environ.get("GMODE", "loop")   # "loop": 20 indirect DMAs; "dg": dma_gather
NSW = 4 if GMODE == "dg" else 1           # SWDGE queues for the edge gathers


def _mmcast(ap):
    """Optionally bitcast an fp32 matmul operand to float32r (fast PE mode)."""
    return ap.bitcast(mybir.dt.float32r) if FP32R else ap

_CACHE = {}


def _gather_q(nc, q, **kw):
    bi = nc.gpsimd.indirect_dma_start(**kw)
    if q:
        bi.ins.queue = f"qPoolDynamic{q}"
    return bi


def _build_layer(nc, tc, g, li, d_in, x2t, x2t_next, w1_d, b1_d, w2_d, b2_d,
                 w3_d, b3_d, v_d, out_d=None):
    """Emit one EdgeConv layer. x2t holds 2*X^T (rows 0..d_in-1 valid).
    Writes 2*X_next^T into x2t_next, or the final output to out_d (layer 3)."""
    sb = g["sb"]
    pp_s, pp_tp, pp_h = g["pp_s"], g["pp_tp"], g["pp_h"]
    ident, ones1, alpha64, alpha128, base = (
        g["ident"], g["ones1"], g["alpha64"], g["alpha128"], g["base"])

    # ---- phase 0: weights ----
    wa = sb.tile([d_in, H], F32, tag="wa")
    wb = sb.tile([d_in, H], F32, tag="wb")
    nc.sync.dma_start(wa[:], w1_d[0:d_in, :])
    nc.sync.dma_start(wb[:], w1_d[d_in:2 * d_in, :])
    wd2 = sb.tile([d_in, H], F32, tag="wd2")
    nc.vector.tensor_tensor(out=wd2[:], in0=wa[:], in1=wb[:], op=ALU.subtract)
    nc.vector.tensor_scalar_mul(wd2[:], wd2[:], 0.5)
    wb2 = sb.tile([d_in, H], F32, tag="wb2")
    nc.vector.tensor_scalar_mul(wb2[:], wb[:], 0.5)
    w2 = sb.tile([H, H], F32, tag="w2")
    nc.sync.dma_start(w2[:], w2_d[:])
    w3 = sb.tile([H, H], F32, tag="w3")
    nc.sync.dma_start(w3[:], w3_d[:])
    b1r = sb.tile([1, H], F32, tag="b1r")
    nc.sync.dma_start(b1r[:], b1_d[:].unsqueeze(0))
    b2c = sb.tile([H, 1], F32, tag="b2c")
    nc.sync.dma_start(b2c[:], b2_d[:].unsqueeze(1))
    b3c = sb.tile([H, 1], F32, tag="b3c")
    nc.sync.dma_start(b3c[:], b3_d[:].unsqueeze(1))

    # ---- phase 0: negsq2 = -2*sq (from X2T: colsum(X2T^2) = 4 sq) ----
    xsq = g["s_pool"].tile([P, N], F32, tag="s")   # borrow an S buffer
    nc.scalar.activation(out=xsq[0:d_in, :], in_=x2t[0:d_in, :], func=AF.Square)
    onescol = sb.tile([d_in, 1], F32, tag="ones")
    nc.vector.memset(onescol[:], 1.0)
    negsq2 = sb.tile([1, N], F32, tag="negsq")
    for c in range(N // 512):
        ps = pp_tp.tile([1, 512], F32, tag="tp")
        nc.tensor.matmul(out=ps[:], lhsT=onescol[:], rhs=xsq[0:d_in, c * 512:(c + 1) * 512],
                         start=True, stop=True)
        nc.scalar.activation(out=negsq2[:, c * 512:(c + 1) * 512], in_=ps[:],
                             func=AF.Copy, scale=-0.5)
    # replicate -2sq across partitions once per layer (PE rank-1), so the
    # per-tile correction is a single wide GPSIMD add instead of 8 K=1 matmuls
    negsq_rep = sb.tile([P, N], F32, tag="negsqrep")
    for c in range(N // 512):
        ps = pp_tp.tile([P, 512], F32, tag="tp")
        nc.tensor.matmul(out=ps[:], lhsT=ones1[:, 0:P],
                         rhs=negsq2[:, c * 512:(c + 1) * 512], start=True, stop=True)
        nc.scalar.copy(out=negsq_rep[:, c * 512:(c + 1) * 512], in_=ps[:])

    # ---- phase 0: U (point-major SBUF) and V (point-major DRAM) ----
    u_sb = sb.tile([P, NT * H], F32, tag="u")
    vbuf = sb.tile([P, NT * H], F32, tag="vbuf")
    for t in range(NT):
        lhs = x2t[0:d_in, t * P:(t + 1) * P]
        pu = pp_tp.tile([P, H], F32, tag="tp")
        nc.tensor.matmul(out=pu[:], lhsT=_mmcast(lhs), rhs=_mmcast(wd2[:]), start=True, stop=False)
        nc.tensor.matmul(out=pu[:], lhsT=ones1[:, 0:P], rhs=b1r[:], start=False, stop=True)
        nc.scalar.copy(out=u_sb[:, t * H:(t + 1) * H], in_=pu[:])
        pv = pp_tp.tile([P, H], F32, tag="tp")
        nc.tensor.matmul(out=pv[:], lhsT=_mmcast(lhs), rhs=_mmcast(wb2[:]), start=True, stop=True)
        nc.scalar.copy(out=vbuf[:, t * H:(t + 1) * H], in_=pv[:])
    nc.sync.dma_start(v_d[:].rearrange("(t p) f -> p t f", p=P),
                      vbuf[:].rearrange("p (t f) -> p t f", f=H))

    if STAGE < 4:
        if out_d is None:
            nc.vector.memset(x2t_next[0:H, :], 0.5)
        else:
            nc.vector.memset(g["vbuf_view"][:], 0.5)
    if STAGE < 1:
        if out_d is not None:
            nc.sync.dma_start(out_d[:].rearrange("(t p) f -> p t f", p=P),
                              g["vbuf_view"][:].rearrange("p (t f) -> p t f", f=H))
        return
    # ---- per row-tile ----
    for t in range(NT):
        # distance tile S [128, N] (bigger = closer)
        s_sb = g["s_pool"].tile([P, N], F32, tag="s")
        for c in range(N // 512):
            ps = pp_s.tile([P, 512], F32, tag="dist")
            nc.tensor.matmul(out=ps[:],
                             lhsT=_mmcast(x2t[0:d_in, t * P:(t + 1) * P]),
                             rhs=_mmcast(x2t[0:d_in, c * 512:(c + 1) * 512]),
                             start=True, stop=True)
            nc.scalar.copy(out=s_sb[:, c * 512:(c + 1) * 512], in_=ps[:])
        nc.gpsimd.tensor_tensor(out=s_sb[:], in0=s_sb[:], in1=negsq_rep[:], op=ALU.add)

        # exact top-24 of each row with in-place knockout (8 wide DVE insts):
        # max8 -> indices -> replace-with-NEG, three rounds. match_replace
        # only zaps already-extracted values, so later max_index positions
        # in the modified S equal positions in the original.
        winners = g["k_pool"].tile([P, 24], F32, tag="win")
        idxt = g["k_pool"].tile([P, 24], U32, tag="idxt")
        if SKIP_TOPK:
            nc.vector.memset(winners[:], 0.0)
            nc.vector.memset(idxt[:], 0)
        else:
            for r in range(3):
                nc.vector.max(out=winners[:, r * 8:(r + 1) * 8], in_=s_sb[:])
                nc.vector.max_index(out=idxt[:, r * 8:(r + 1) * 8],
                                    in_max=winners[:, r * 8:(r + 1) * 8],
                                    in_values=s_sb[:])
                if r < 2:
                    nc.vector.match_replace(out=s_sb[:],
                                            in_to_replace=winners[:, r * 8:(r + 1) * 8],
                                            in_values=s_sb[:], imm_value=NEG)

        if STAGE < 2:
            continue
        # edge features: h1 = leaky(U_i + V_j)
        h1 = g["h1_pool"].tile([P, K * H], F32, tag="h1")
        h1v = h1[:].rearrange("p (k f) -> p k f", k=K)
        u_bc = u_sb[:, t * H:(t + 1) * H].unsqueeze(1).to_broadcast([P, K, H])
        if GMODE == "dg" and not SKIP_GATHER:
            # Build the wrapped int16 index layout for dma_gather: gathered
            # position e = k*128 + p must sit at partition e%16 = p%16,
            # slot e//16 = 8k + p//16. Selection matmuls move idxt[16h+q, k]
            # -> pw[q, 8k+h]; one more matmul replicates 16 -> 128 parts.
            idx_f = g["k_pool"].tile([P, 24], F32, tag="idxf")
            nc.vector.tensor_copy(out=idx_f[:], in_=idxt[:])
            pwr = pp_tp.tile([P, 160], F32, tag="pwr")
            pwv = pwr[0:16, :].rearrange("q (k h) -> q h k", h=8)
            for h in range(8):
                nc.tensor.matmul(out=pwv[:, h:h + 1, :],
                                 lhsT=ident[:, 16 * h:16 * h + 16],
                                 rhs=idx_f[:, 0:K], start=True, stop=True)
            w16 = g["k_pool"].tile([16, 160], F32, tag="w16")
            nc.scalar.copy(out=w16[:], in_=pwr[0:16, :])
            nc.tensor.matmul(out=pwr[:], lhsT=g["rep16"][:], rhs=w16[:],
                             start=True, stop=True)
            widx = g["k_pool"].tile([P, 160], I16, tag="widx")
            nc.vector.tensor_copy(out=widx[:], in_=pwr[:])
            nc.gpsimd.dma_gather(
                out_ap=h1v, in_ap=v_d[:], idxs_ap=widx[:],
                num_idxs=K * P, num_idxs_reg=K * P, elem_size=H,
                queue_num=t % NSW, single_packet=False)
            nc.vector.tensor_tensor(out=h1v, in0=h1v, in1=u_bc, op=ALU.add)
        else:
            nc.scalar.copy(out=h1v, in_=u_bc)
            for k in (range(0) if SKIP_GATHER else range(K)):
                _gather_q(nc, k % NSW,
                          out=h1[:, k * H:(k + 1) * H], out_offset=None,
                          in_=v_d[:],
                          in_offset=bass.IndirectOffsetOnAxis(ap=idxt[:, k:k + 1], axis=0),
                          compute_op=ALU.add)
        if SIM_ACT:
            nc.scalar.activation(out=h1[:], in_=h1[:], func=AF.Copy)
        else:
            nc.scalar.activation(out=h1[:], in_=h1[:], func=AF.Prelu, alpha=alpha128[:])

        if STAGE < 3:
            continue
        # transpose to edge-major h1T [64, k*128+i]
        h1t = g["ht_pool"].tile([H, K * P], F32, tag="ht")
        for kc in range(5):
            pt = pp_tp.tile([H, 512], F32, tag="tp")
            for j in range(4):
                k = kc * 4 + j
                nc.tensor.transpose(out=pt[:, j * P:(j + 1) * P],
                                    in_=h1[:, k * H:(k + 1) * H], identity=ident[:])
            nc.scalar.copy(out=h1t[:, kc * 512:(kc + 1) * 512], in_=pt[:])

        if STAGE < 4:
            continue
        # MLP layers 2, 3 (feature-major, edges on the free axis)
        h2t = g["ht_pool"].tile([H, K * P], F32, tag="ht")
        for e in range(5):
            ph = pp_h.tile([H, 512], F32, tag="h")
            nc.tensor.matmul(out=ph[:], lhsT=_mmcast(w2[:]), rhs=_mmcast(h1t[:, e * 512:(e + 1) * 512]),
                             start=True, stop=True)
            nc.scalar.activation(out=h2t[:, e * 512:(e + 1) * 512], in_=ph[:],
                                 func=AF.Copy if SIM_ACT else AF.Prelu,
                                 bias=0.0 if SIM_ACT else b2c[:],
                                 alpha=0.0 if SIM_ACT else alpha64[:])
        h3t = g["ht_pool"].tile([H, K * P], F32, tag="ht")
        for e in range(5):
            ph = pp_h.tile([H, 512], F32, tag="h")
            nc.tensor.matmul(out=ph[:], lhsT=_mmcast(w3[:]), rhs=_mmcast(h2t[:, e * 512:(e + 1) * 512]),
                             start=True, stop=True)
            nc.scalar.activation(out=h3t[:, e * 512:(e + 1) * 512], in_=ph[:],
                                 func=AF.Copy if SIM_ACT else AF.Prelu,
                                 bias=0.0 if SIM_ACT else b3c[:],
                                 alpha=0.0 if SIM_ACT else alpha64[:])

        # aggregate: max over k (innermost stride-128 axis)
        ftile = g["f_pool"].tile([H, P], F32, tag="f")
        nc.vector.tensor_reduce(out=ftile[:],
                                in_=h3t[:].rearrange("h (k i) -> h i k", k=K),
                                axis=mybir.AxisListType.X, op=ALU.max)
        if out_d is None:
            nc.scalar.mul(out=x2t_next[0:H, t * P:(t + 1) * P], in_=ftile[:], mul=2.0)
        else:
            po = pp_tp.tile([P, H], F32, tag="tp")
            nc.tensor.transpose(out=po[:], in_=ftile[:], identity=ident[0:H, 0:H])
            nc.scalar.copy(out=g["vbuf_view"][:, t * H:(t + 1) * H], in_=po[:])
    if out_d is not None:
        nc.sync.dma_start(out_d[:].rearrange("(t p) f -> p t f", p=P),
                          g["vbuf_view"][:].rearrange("p (t f) -> p t f", f=H))


# weight-blob layout: per layer w1, w2, w3, b1, b2, b3 (fp32 elements).
# Packing all 18 weight tensors into ONE device input matters: each extra
# input tensor costs ~1.5 ms of per-call host/axon binding overhead.
def _blob_layout():
    off, lay = 0, []
    for li in range(3):
        d2 = 6 if li == 0 else 128
        ent = {}
        for nm, shp in ((f"w{li+1}1", (d2, H)), (f"w{li+1}2", (H, H)),
                        (f"w{li+1}3", (H, H)), (f"b{li+1}1", (H,)),
                        (f"b{li+1}2", (H,)), (f"b{li+1}3", (H,))):
            n = int(np.prod(shp))
            ent[nm] = (off, shp)
            off += n
        lay.append(ent)
    return lay, off


def build():
    nc = bacc.Bacc("TRN2", target_bir_lowering=False, debug=False,
                   num_swdge_queues=NSW)
    pos_d = nc.dram_tensor("pos", [N, 3], F32, kind="ExternalInput")
    lay, tot = _blob_layout()
    wblob_d = nc.dram_tensor("wblob", [tot], F32, kind="ExternalInput")
    wnames = {}
    for li in range(3):
        for nm, (off, shp) in lay[li].items():
            n = int(np.prod(shp))
            v = wblob_d[off:off + n]
            if len(shp) == 2:
                v = v.rearrange("(r c) -> r c", c=shp[1])
            wnames[nm] = v
    out_d = nc.dram_tensor("out", [N, H], F32, kind="ExternalOutput")
    v_ds = [nc.dram_tensor(f"vtab{li}", [N, H], F32) for li in range(3)]

    with tile.TileContext(nc) as tc:
        with tc.tile_pool(name="sb", bufs=1) as sb, \
             tc.tile_pool(name="s_pool", bufs=2) as s_pool, \
             tc.tile_pool(name="k_pool", bufs=2) as k_pool, \
             tc.tile_pool(name="h1_pool", bufs=2) as h1_pool, \
             tc.tile_pool(name="ht_pool", bufs=4) as ht_pool, \
             tc.tile_pool(name="f_pool", bufs=2) as f_pool, \
             tc.tile_pool(name="pp_s", bufs=2, space="PSUM") as pp_s, \
             tc.tile_pool(name="pp_tp", bufs=2, space="PSUM") as pp_tp, \
             tc.tile_pool(name="pp_h", bufs=2, space="PSUM") as pp_h:

            g = dict(sb=sb, s_pool=s_pool, k_pool=k_pool, h1_pool=h1_pool,
                     ht_pool=ht_pool, f_pool=f_pool,
                     pp_s=pp_s, pp_tp=pp_tp, pp_h=pp_h)

            ident = sb.tile([P, P], F32, tag="ident")
            make_identity(nc, ident)
            g["ident"] = ident
            ones1 = sb.tile([1, P], F32, tag="ones1")
            nc.vector.memset(ones1[:], 1.0)
            g["ones1"] = ones1
            alpha64 = sb.tile([H, 1], F32, tag="alpha64")
            nc.vector.memset(alpha64[:], SLOPE)
            g["alpha64"] = alpha64
            alpha128 = sb.tile([P, 1], F32, tag="alpha128")
            nc.vector.memset(alpha128[:], SLOPE)
            g["alpha128"] = alpha128
            base = sb.tile([P, NCH * 8], U32, tag="base")
            nc.gpsimd.iota(base[:], pattern=[[CS, NCH], [0, 8]], base=0,
                           channel_multiplier=0)
            g["base"] = base
            # rep16[q, p] = 1 if p % 16 == q: replicates a 16-partition tile
            # to all 128 partitions via one matmul.
            rep16 = sb.tile([16, P], F32, tag="rep16")
            nc.vector.memset(rep16[:], 1.0)
            nc.gpsimd.affine_select(
                out=rep16[:].rearrange("q (a r) -> q a r", r=16),
                in_=rep16[:].rearrange("q (a r) -> q a r", r=16),
                compare_op=ALU.is_equal, fill=0.0, base=0,
                pattern=[[0, 8], [1, 16]], channel_multiplier=-1)
            g["rep16"] = rep16

            # layer inputs: 2*X^T ping-pong
            x2t_a = sb.tile([H, N], F32, tag="x2t_a")
            x2t_b = sb.tile([H, N], F32, tag="x2t_b")
            vbuf_view = sb.tile([P, NT * H], F32, tag="obuf")
            g["vbuf_view"] = vbuf_view

            # load pos -> 2*X^T (rows 0..2)
            xsb = sb.tile([P, NT * 3], F32, tag="xsb")
            nc.sync.dma_start(xsb[:].rearrange("p (t d) -> p t d", d=3),
                              pos_d[:].rearrange("(t p) d -> p t d", p=P))
            for t in range(NT):
                pt = pp_tp.tile([3, P], F32, tag="tp")
                nc.tensor.transpose(out=pt[:], in_=xsb[:, t * 3:(t + 1) * 3],
                                    identity=ident[:])
                nc.scalar.mul(out=x2t_a[0:3, t * P:(t + 1) * P], in_=pt[:], mul=2.0)

            _build_layer(nc, tc, g, 0, 3, x2t_a, x2t_b,
                         wnames["w11"], wnames["b11"], wnames["w12"], wnames["b12"],
                         wnames["w13"], wnames["b13"], v_ds[0])
            _build_layer(nc, tc, g, 1, H, x2t_b, x2t_a,
                         wnames["w21"], wnames["b21"], wnames["w22"], wnames["b22"],
                         wnames["w23"], wnames["b23"], v_ds[1])
            _build_layer(nc, tc, g, 2, H, x2t_a, None,
                         wnames["w31"], wnames["b31"], wnames["w32"], wnames["b32"],
                         wnames["w33"], wnames["b33"], v_ds[2], out_d=out_d)
    nc.finalize()
    return nc


def pack_wblob(inputs):
    lay, tot = _blob_layout()
    blob = np.empty(tot, np.float32)
    for li in range(3):
        for nm, (off, shp) in lay[li].items():
            a = np.asarray(inputs[nm], dtype=np.float32).reshape(-1)
            blob[off:off + a.size] = a
    return blob


def make_in_maps(inputs):
    pos = np.ascontiguousarray(np.asarray(inputs["pos"], dtype=np.float32))
    blob = pack_wblob(inputs)
    return [{"pos": pos[b], "wblob": blob} for b in range(B)]


def _make_runner(nc):
    """Cached jitted NEFF executor: warm kernel() calls skip retracing.
    The weight blob is passed replicated (one host copy, not an 8x concat)."""
    import jax
    from jax.sharding import Mesh, PartitionSpec
    from jax.experimental.shard_map import shard_map
    from concourse.bass2jax import (_bass_exec_p, install_neuronx_cc_hook,
                                    partition_id_tensor)
    install_neuronx_cc_hook()
    partition_name = nc.partition_id_tensor.name if nc.partition_id_tensor else None
    in_names, out_names, out_avals, zero_shapes = [], [], [], []
    for alloc in nc.m.functions[0].allocations:
        if not isinstance(alloc, mybir.MemoryLocationSet):
            continue
        name = alloc.memorylocations[0].name
        if alloc.kind == "ExternalInput":
            if name != partition_name:
                in_names.append(name)
        elif alloc.kind == "ExternalOutput":
            out_names.append(name)
            shape = tuple(alloc.tensor_shape)
            dtype = mybir.dt.np(alloc.dtype)
            out_avals.append(jax.core.ShapedArray(shape, dtype))
            zero_shapes.append((shape, dtype))
    n_params = len(in_names)
    n_outs = len(out_avals)
    in_names_all = list(in_names) + out_names
    if partition_name is not None:
        in_names_all.append(partition_name)

    def _body(*args):
        operands = list(args)
        if partition_name is not None:
            operands.append(partition_id_tensor())
        return tuple(_bass_exec_p.bind(
            *operands, out_avals=tuple(out_avals),
            in_names=tuple(in_names_all), out_names=tuple(out_names),
            lowering_input_output_aliases=(),
            sim_require_finite=True, sim_require_nnan=True, nc=nc))

    devices = jax.devices()[:B]
    assert len(devices) >= B or len(devices) == B
    mesh = Mesh(np.asarray(devices), ("core",))
    rep_names = {"wblob"} & set(in_names)
    in_specs = tuple(
        PartitionSpec() if nm in rep_names else PartitionSpec("core")
        for nm in in_names
    ) + (PartitionSpec("core"),) * n_outs
    sharded = jax.jit(
        shard_map(_body, mesh=mesh, in_specs=in_specs,
                  out_specs=(PartitionSpec("core"),) * len(out_names),
                  check_rep=False),
        donate_argnums=tuple(range(n_params, n_params + n_outs)),
        keep_unused=True,
    )

    def run(in_maps):
        per_core = [[np.asarray(m[name]) for name in in_names] for m in in_maps]
        args_in = [
            per_core[0][i] if in_names[i] in rep_names else
            np.concatenate([per_core[c][i] for c in range(B)], axis=0)
            for i in range(n_params)
        ]
        zeros = [np.zeros((B * sh[0], *sh[1:]), dt) for sh, dt in zero_shapes]
        out_arrs = sharded(*args_in, *zeros)
        jax.block_until_ready(out_arrs)
        return [
            {name: np.asarray(out_arrs[i]).reshape(B, *out_avals[i].shape)[c]
             for i, name in enumerate(out_names)}
            for c in range(B)
        ]

    return run


def kernel(**inputs):
    if "nc" not in _CACHE:
        _CACHE["nc"] = build()
        _CACHE["run"] = _make_runner(_CACHE["nc"])
    in_maps = make_in_maps(inputs)
    results = _CACHE["run"](in_maps)
    out = np.stack([results[b]["out"] for b in range(B)], axis=0)
    return out


if __name__ == "__main__":
    rng = np.random.default_rng(0)
    fake = {"pos": rng.standard_normal((B, N, 3)).astype(np.float32)}
    for pfx in ("1", "2", "3"):
        d2 = 6 if pfx == "1" else 128
        fake[f"w{pfx}1"] = rng.standard_normal((d2, H)).astype(np.float32) * 0.2
        fake[f"w{pfx}2"] = rng.standard_normal((H, H)).astype(np.float32) * 0.12
        fake[f"w{pfx}3"] = rng.standard_normal((H, H)).astype(np.float32) * 0.12
        for j in ("1", "2", "3"):
            fake[f"b{pfx}{j}"] = np.zeros(H, np.float32)
    o = kernel(**fake)
    print("out", o.shape, o.dtype, float(np.abs(o).max()))



# revision 2
# speedup vs baseline: 1.5108x; 1.5108x over previous
"""DGCNN (3x DynamicEdgeConv, kNN=20) Trainium2 Bass kernel.

Self-contained: `kernel(**inputs) -> np.ndarray` takes the full inputs from
setup_inputs() (pos [8,4096,3] + 9 weight/bias pairs) and returns [8,4096,64].

Sharding: data-parallel over batch B=8 -> one point cloud per NeuronCore,
weights replicated. Each core runs the identical program on its slice.

Per-core, per-layer pipeline (N=4096 points, feature dim D in {3,64}, H=64):
  phase 0: X2T = 2*X^T in SBUF [64,4096]; negsq2 = -2*||x_j||^2 row;
           U = x@(W1a-W1b)+b1 (point-major, SBUF), V = x@W1b -> DRAM [4096,64]
  per row-tile t (128 points):
    S = 4 x_i.x_j - 2 sq_j  (PE, accumulating -2sq row; row-monotone == -dist)
    top-20: per-128-chunk top-8 via DVE max/max_index -> 256 candidates,
            3x (max8 + match_replace) -> 24 winners, positions via max_index,
            per-row candidate-index lookup via gpsimd indirect_copy + 16
            strided-partition diagonal DMAs -> idx [128,24] (first 20 valid)
    h1 = leaky(U_i + V_j): prefill U (ACT), 20x indirect DMA gather with
            CCE-add, Prelu(alpha=0.2)
    h1 -> 20 PE transposes -> h1T [64, 20*128] (edge k-major)
    h2T = Prelu(W2^T@h1T + b2), h3T = Prelu(W3^T@h2T + b3)   (PE + ACT)
    out tile = max over k (DVE strided reduce) -> next layer's X2T
"""
import numpy as np

import concourse.bass as bass
import concourse.bacc as bacc
import concourse.mybir as mybir
import concourse.tile as tile
from concourse.bass_utils import run_bass_kernel_spmd
from concourse.masks import make_identity

F32 = mybir.dt.float32
U16 = mybir.dt.uint16
U32 = mybir.dt.uint32
I16 = mybir.dt.int16
AF = mybir.ActivationFunctionType
ALU = mybir.AluOpType

B = 8
N = 4096
P = 128
NT = N // P            # 32 row tiles
CS = 128               # top-k chunk size
NCH = N // CS          # 32 chunks
K = 20
H = 64
SLOPE = 0.2
NEG = -3.0e38
import os
SKIP_GATHER = bool(int(os.environ.get("SKIP_GATHER", "0")))
SKIP_TOPK = bool(int(os.environ.get("SKIP_TOPK", "0")))
FP32R = bool(int(os.environ.get("FP32R", "0")))
STAGE = int(os.environ.get("STAGE", "4"))  # truncate per-tile pipeline for attribution
SIM_ACT = bool(int(os.environ.get("SIM_ACT", "0")))  # Prelu->Copy for CoreSim
GMODE = os.environ.get("GMODE", "loop")   # "loop": 20 indirect DMAs; "dg": dma_gather
NSW = 4 if GMODE == "dg" else 1           # SWDGE queues for the edge gathers


def _mmcast(ap):
    """Optionally bitcast an fp32 matmul operand to float32r (fast PE mode)."""
    return ap.bitcast(mybir.dt.float32r) if FP32R else ap

_CACHE = {}


def _gather_q(nc, q, **kw):
    bi = nc.gpsimd.indirect_dma_start(**kw)
    if q:
        bi.ins.queue = f"qPoolDynamic{q}"
    return bi


def _build_layer(nc, tc, g, li, d_in, x2t, x2t_next, w1_d, b1_d, w2_d, b2_d,
                 w3_d, b3_d, v_d, out_d=None):
    """Emit one EdgeConv layer. x2t holds 2*X^T (rows 0..d_in-1 valid).
    Writes 2*X_next^T into x2t_next, or the final output to out_d (layer 3)."""
    sb = g["sb"]
    pp_s, pp_tp, pp_h = g["pp_s"], g["pp_tp"], g["pp_h"]
    ident, ones1, alpha64, alpha128, base = (
        g["ident"], g["ones1"], g["alpha64"], g["alpha128"], g["base"])

    # ---- phase 0: weights ----
    wa = sb.tile([d_in, H], F32, tag="wa")
    wb = sb.tile([d_in, H], F32, tag="wb")
    nc.sync.dma_start(wa[:], w1_d[0:d_in, :])
    nc.sync.dma_start(wb[:], w1_d[d_in:2 * d_in, :])
    wd2 = sb.tile([d_in, H], F32, tag="wd2")
    nc.vector.tensor_tensor(out=wd2[:], in0=wa[:], in1=wb[:], op=ALU.subtract)
    nc.vector.tensor_scalar_mul(wd2[:], wd2[:], 0.5)
    wb2 = sb.tile([d_in, H], F32, tag="wb2")
    nc.vector.tensor_scalar_mul(wb2[:], wb[:], 0.5)
    w2 = sb.tile([H, H], F32, tag="w2")
    nc.sync.dma_start(w2[:], w2_d[:])
    w3 = sb.tile([H, H], F32, tag="w3")
    nc.sync.dma_start(w3[:], w3_d[:])
    b1r = sb.tile([1, H], F32, tag="b1r")
    nc.sync.dma_start(b1r[:], b1_d[:].unsqueeze(0))
    b2c = sb.tile([H, 1], F32, tag="b2c")
    nc.sync.dma_start(b2c[:], b2_d[:].unsqueeze(1))
    b3c = sb.tile([H, 1], F32, tag="b3c")
    nc.sync.dma_start(b3c[:], b3_d[:].unsqueeze(1))

    # ---- phase 0: negsq2 = -2*sq (from X2T: colsum(X2T^2) = 4 sq) ----
    xsq = g["s_pool"].tile([P, N], F32, tag="s")   # borrow an S buffer
    nc.scalar.activation(out=xsq[0:d_in, :], in_=x2t[0:d_in, :], func=AF.Square)
    onescol = sb.tile([d_in, 1], F32, tag="ones")
    nc.vector.memset(onescol[:], 1.0)
    negsq2 = sb.tile([1, N], F32, tag="negsq")
    for c in range(N // 512):
        ps = pp_tp.tile([1, 512], F32, tag="tp")
        nc.tensor.matmul(out=ps[:], lhsT=onescol[:], rhs=xsq[0:d_in, c * 512:(c + 1) * 512],
                         start=True, stop=True)
        nc.scalar.activation(out=negsq2[:, c * 512:(c + 1) * 512], in_=ps[:],
                             func=AF.Copy, scale=-0.5)
    # replicate -2sq across partitions once per layer (PE rank-1), so the
    # per-tile correction is a single wide GPSIMD add instead of 8 K=1 matmuls
    negsq_rep = sb.tile([P, N], F32, tag="negsqrep")
    for c in range(N // 512):
        ps = pp_tp.tile([P, 512], F32, tag="tp")
        nc.tensor.matmul(out=ps[:], lhsT=ones1[:, 0:P],
                         rhs=negsq2[:, c * 512:(c + 1) * 512], start=True, stop=True)
        nc.scalar.copy(out=negsq_rep[:, c * 512:(c + 1) * 512], in_=ps[:])

    # ---- phase 0: U (point-major SBUF) and V (point-major DRAM) ----
    u_sb = sb.tile([P, NT * H], F32, tag="u")
    vbuf = sb.tile([P, NT * H], F32, tag="vbuf")
    for t in range(NT):
        lhs = x2t[0:d_in, t * P:(t + 1) * P]
        pu = pp_tp.tile([P, H], F32, tag="tp")
        nc.tensor.matmul(out=pu[:], lhsT=_mmcast(lhs), rhs=_mmcast(wd2[:]), start=True, stop=False)
        nc.tensor.matmul(out=pu[:], lhsT=ones1[:, 0:P], rhs=b1r[:], start=False, stop=True)
        nc.scalar.copy(out=u_sb[:, t * H:(t + 1) * H], in_=pu[:])
        pv = pp_tp.tile([P, H], F32, tag="tp")
        nc.tensor.matmul(out=pv[:], lhsT=_mmcast(lhs), rhs=_mmcast(wb2[:]), start=True, stop=True)
        nc.scalar.copy(out=vbuf[:, t * H:(t + 1) * H], in_=pv[:])
    nc.sync.dma_start(v_d[:].rearrange("(t p) f -> p t f", p=P),
                      vbuf[:].rearrange("p (t f) -> p t f", f=H))

    if STAGE < 4:
        if out_d is None:
            nc.vector.memset(x2t_next[0:H, :], 0.5)
        else:
            nc.vector.memset(g["vbuf_view"][:], 0.5)
    if STAGE < 1:
        if out_d is not None:
            nc.sync.dma_start(out_d[:].rearrange("(t p) f -> p t f", p=P),
                              g["vbuf_view"][:].rearrange("p (t f) -> p t f", f=H))
        return
    # ---- per row-tile ----
    for t in range(NT):
        # distance tile S [128, N] (bigger = closer)
        s_sb = g["s_pool"].tile([P, N], F32, tag="s")
        for c in range(N // 512):
            ps = pp_s.tile([P, 512], F32, tag="dist")
            nc.tensor.matmul(out=ps[:],
                             lhsT=_mmcast(x2t[0:d_in, t * P:(t + 1) * P]),
                             rhs=_mmcast(x2t[0:d_in, c * 512:(c + 1) * 512]),
                             start=True, stop=True)
            nc.scalar.copy(out=s_sb[:, c * 512:(c + 1) * 512], in_=ps[:])
        nc.gpsimd.tensor_tensor(out=s_sb[:], in0=s_sb[:], in1=negsq_rep[:], op=ALU.add)

        # exact top-24 of each row with in-place knockout (8 wide DVE insts):
        # max8 -> indices -> replace-with-NEG, three rounds. match_replace
        # only zaps already-extracted values, so later max_index positions
        # in the modified S equal positions in the original.
        winners = g["k_pool"].tile([P, 24], F32, tag="win")
        idxt = g["k_pool"].tile([P, 24], U32, tag="idxt")
        if SKIP_TOPK:
            nc.vector.memset(winners[:], 0.0)
            nc.vector.memset(idxt[:], 0)
        else:
            for r in range(3):
                nc.vector.max(out=winners[:, r * 8:(r + 1) * 8], in_=s_sb[:])
                nc.vector.max_index(out=idxt[:, r * 8:(r + 1) * 8],
                                    in_max=winners[:, r * 8:(r + 1) * 8],
                                    in_values=s_sb[:])
                if r < 2:
                    nc.vector.match_replace(out=s_sb[:],
                                            in_to_replace=winners[:, r * 8:(r + 1) * 8],
                                            in_values=s_sb[:], imm_value=NEG)

        if STAGE < 2:
            continue
        # edge features: h1 = leaky(U_i + V_j)
        h1 = g["h1_pool"].tile([P, K * H], F32, tag="h1")
        h1v = h1[:].rearrange("p (k f) -> p k f", k=K)
        u_bc = u_sb[:, t * H:(t + 1) * H].unsqueeze(1).to_broadcast([P, K, H])
        if GMODE == "dg" and not SKIP_GATHER:
            # Build the wrapped int16 index layout for dma_gather: gathered
            # position e = k*128 + p must sit at partition e%16 = p%16,
            # slot e//16 = 8k + p//16. Selection matmuls move idxt[16h+q, k]
            # -> pw[q, 8k+h]; one more matmul replicates 16 -> 128 parts.
            idx_f = g["k_pool"].tile([P, 24], F32, tag="idxf")
            nc.vector.tensor_copy(out=idx_f[:], in_=idxt[:])
            pwr = pp_tp.tile([P, 160], F32, tag="pwr")
            pwv = pwr[0:16, :].rearrange("q (k h) -> q h k", h=8)
            for h in range(8):
                nc.tensor.matmul(out=pwv[:, h:h + 1, :],
                                 lhsT=ident[:, 16 * h:16 * h + 16],
                                 rhs=idx_f[:, 0:K], start=True, stop=True)
            w16 = g["k_pool"].tile([16, 160], F32, tag="w16")
            nc.scalar.copy(out=w16[:], in_=pwr[0:16, :])
            nc.tensor.matmul(out=pwr[:], lhsT=g["rep16"][:], rhs=w16[:],
                             start=True, stop=True)
            widx = g["k_pool"].tile([P, 160], I16, tag="widx")
            nc.vector.tensor_copy(out=widx[:], in_=pwr[:])
            nc.gpsimd.dma_gather(
                out_ap=h1v, in_ap=v_d[:], idxs_ap=widx[:],
                num_idxs=K * P, num_idxs_reg=K * P, elem_size=H,
                queue_num=t % NSW, single_packet=False)
            nc.vector.tensor_tensor(out=h1v, in0=h1v, in1=u_bc, op=ALU.add)
        else:
            nc.scalar.copy(out=h1v, in_=u_bc)
            for k in (range(0) if SKIP_GATHER else range(K)):
                _gather_q(nc, k % NSW,
                          out=h1[:, k * H:(k + 1) * H], out_offset=None,
                          in_=v_d[:],
                          in_offset=bass.IndirectOffsetOnAxis(ap=idxt[:, k:k + 1], axis=0),
                          compute_op=ALU.add)
        if SIM_ACT:
            nc.scalar.activation(out=h1[:], in_=h1[:], func=AF.Copy)
        else:
            nc.scalar.activation(out=h1[:], in_=h1[:], func=AF.Prelu, alpha=alpha128[:])

        if STAGE < 3:
            continue
        # transpose to edge-major h1T [64, k*128+i]
        h1t = g["ht_pool"].tile([H, K * P], F32, tag="ht")
        for kc in range(5):
            pt = pp_tp.tile([H, 512], F32, tag="tp")
            for j in range(4):
                k = kc * 4 + j
                nc.tensor.transpose(out=pt[:, j * P:(j + 1) * P],
                                    in_=h1[:, k * H:(k + 1) * H], identity=ident[:])
            nc.scalar.copy(out=h1t[:, kc * 512:(kc + 1) * 512], in_=pt[:])

        if STAGE < 4:
            continue
        # MLP layers 2, 3 (feature-major, edges on the free axis)
        h2t = g["ht_pool"].tile([H, K * P], F32, tag="ht")
        for e in range(5):
            ph = pp_h.tile([H, 512], F32, tag="h")
            nc.tensor.matmul(out=ph[:], lhsT=_mmcast(w2[:]), rhs=_mmcast(h1t[:, e * 512:(e + 1) * 512]),
                             start=True, stop=True)
            nc.scalar.activation(out=h2t[:, e * 512:(e + 1) * 512], in_=ph[:],
                                 func=AF.Copy if SIM_ACT else AF.Prelu,
                                 bias=0.0 if SIM_ACT else b2c[:],
                                 alpha=0.0 if SIM_ACT else alpha64[:])
        h3t = g["ht_pool"].tile([H, K * P], F32, tag="ht")
        for e in range(5):
            ph = pp_h.tile([H, 512], F32, tag="h")
            nc.tensor.matmul(out=ph[:], lhsT=_mmcast(w3[:]), rhs=_mmcast(h2t[:, e * 512:(e + 1) * 512]),
                             start=True, stop=True)
            nc.scalar.activation(out=h3t[:, e * 512:(e + 1) * 512], in_=ph[:],
                                 func=AF.Copy if SIM_ACT else AF.Prelu,
                                 bias=0.0 if SIM_ACT else b3c[:],
                                 alpha=0.0 if SIM_ACT else alpha64[:])

        # aggregate: max over k (innermost stride-128 axis)
        ftile = g["f_pool"].tile([H, P], F32, tag="f")
        nc.vector.tensor_reduce(out=ftile[:],
                                in_=h3t[:].rearrange("h (k i) -> h i k", k=K),
                                axis=mybir.AxisListType.X, op=ALU.max)
        if out_d is None:
            nc.scalar.mul(out=x2t_next[0:H, t * P:(t + 1) * P], in_=ftile[:], mul=2.0)
        else:
            po = pp_tp.tile([P, H], F32, tag="tp")
            nc.tensor.transpose(out=po[:], in_=ftile[:], identity=ident[0:H, 0:H])
            nc.scalar.copy(out=g["vbuf_view"][:, t * H:(t + 1) * H], in_=po[:])
    if out_d is not None:
        nc.sync.dma_start(out_d[:].rearrange("(t p) f -> p t f", p=P),
                          g["vbuf_view"][:].rearrange("p (t f) -> p t f", f=H))


# weight-blob layout: per layer w1, w2, w3, b1, b2, b3 (fp32 elements).
# Packing all 18 weight tensors into ONE device input matters: each extra
# input tensor costs ~1.5 ms of per-call host/axon binding overhead.
def _blob_layout():
    off, lay = 0, []
    for li in range(3):
        d2 = 6 if li == 0 else 128
        ent = {}
        for nm, shp in ((f"w{li+1}1", (d2, H)), (f"w{li+1}2", (H, H)),
                        (f"w{li+1}3", (H, H)), (f"b{li+1}1", (H,)),
                        (f"b{li+1}2", (H,)), (f"b{li+1}3", (H,))):
            n = int(np.prod(shp))
            ent[nm] = (off, shp)
            off += n
        lay.append(ent)
    return lay, off


def build():
    nc = bacc.Bacc("TRN2", target_bir_lowering=False, debug=False,
                   num_swdge_queues=NSW)
    pos_d = nc.dram_tensor("pos", [N, 3], F32, kind="ExternalInput")
    lay, tot = _blob_layout()
    wblob_d = nc.dram_tensor("wblob", [tot], F32, kind="ExternalInput")
    wnames = {}
    for li in range(3):
        for nm, (off, shp) in lay[li].items():
            n = int(np.prod(shp))
            v = wblob_d[off:off + n]
            if len(shp) == 2:
                v = v.rearrange("(r c) -> r c", c=shp[1])
            wnames[nm] = v
    out_d = nc.dram_tensor("out", [N, H], F32, kind="ExternalOutput")
    v_ds = [nc.dram_tensor(f"vtab{li}", [N, H], F32) for li in range(3)]

    with tile.TileContext(nc) as tc:
        with tc.tile_pool(name="sb", bufs=1) as sb, \
             tc.tile_pool(name="s_pool", bufs=2) as s_pool, \
             tc.tile_pool(name="k_pool", bufs=2) as k_pool, \
             tc.tile_pool(name="h1_pool", bufs=2) as h1_pool, \
             tc.tile_pool(name="ht_pool", bufs=4) as ht_pool, \
             tc.tile_pool(name="f_pool", bufs=2) as f_pool, \
             tc.tile_pool(name="pp_s", bufs=2, space="PSUM") as pp_s, \
             tc.tile_pool(name="pp_tp", bufs=2, space="PSUM") as pp_tp, \
             tc.tile_pool(name="pp_h", bufs=2, space="PSUM") as pp_h:

            g = dict(sb=sb, s_pool=s_pool, k_pool=k_pool, h1_pool=h1_pool,
                     ht_pool=ht_pool, f_pool=f_pool,
                     pp_s=pp_s, pp_tp=pp_tp, pp_h=pp_h)

            ident = sb.tile([P, P], F32, tag="ident")
            make_identity(nc, ident)
            g["ident"] = ident
            ones1 = sb.tile([1, P], F32, tag="ones1")
            nc.vector.memset(ones1[:], 1.0)
            g["ones1"] = ones1
            alpha64 = sb.tile([H, 1], F32, tag="alpha64")
            nc.vector.memset(alpha64[:], SLOPE)
            g["alpha64"] = alpha64
            alpha128 = sb.tile([P, 1], F32, tag="alpha128")
            nc.vector.memset(alpha128[:], SLOPE)
            g["alpha128"] = alpha128
            base = sb.tile([P, NCH * 8], U32, tag="base")
            nc.gpsimd.iota(base[:], pattern=[[CS, NCH], [0, 8]], base=0,
                           channel_multiplier=0)
            g["base"] = base
            # rep16[q, p] = 1 if p % 16 == q: replicates a 16-partition tile
            # to all 128 partitions via one matmul.
            rep16 = sb.tile([16, P], F32, tag="rep16")
            nc.vector.memset(rep16[:], 1.0)
            nc.gpsimd.affine_select(
                out=rep16[:].rearrange("q (a r) -> q a r", r=16),
                in_=rep16[:].rearrange("q (a r) -> q a r", r=16),
                compare_op=ALU.is_equal, fill=0.0, base=0,
                pattern=[[0, 8], [1, 16]], channel_multiplier=-1)
            g["rep16"] = rep16

            # layer inputs: 2*X^T ping-pong
            x2t_a = sb.tile([H, N], F32, tag="x2t_a")
            x2t_b = sb.tile([H, N], F32, tag="x2t_b")
            vbuf_view = sb.tile([P, NT * H], F32, tag="obuf")
            g["vbuf_view"] = vbuf_view

            # load pos -> 2*X^T (rows 0..2)
            xsb = sb.tile([P, NT * 3], F32, tag="xsb")
            nc.sync.dma_start(xsb[:].rearrange("p (t d) -> p t d", d=3),
                              pos_d[:].rearrange("(t p) d -> p t d", p=P))
            for t in range(NT):
                pt = pp_tp.tile([3, P], F32, tag="tp")
                nc.tensor.transpose(out=pt[:], in_=xsb[:, t * 3:(t + 1) * 3],
                                    identity=ident[:])
                nc.scalar.mul(out=x2t_a[0:3, t * P:(t + 1) * P], in_=pt[:], mul=2.0)

            _build_layer(nc, tc, g, 0, 3, x2t_a, x2t_b,
                         wnames["w11"], wnames["b11"], wnames["w12"], wnames["b12"],
                         wnames["w13"], wnames["b13"], v_ds[0])
            _build_layer(nc, tc, g, 1, H, x2t_b, x2t_a,
                         wnames["w21"], wnames["b21"], wnames["w22"], wnames["b22"],
                         wnames["w23"], wnames["b23"], v_ds[1])
            _build_layer(nc, tc, g, 2, H, x2t_a, None,
                         wnames["w31"], wnames["b31"], wnames["w32"], wnames["b32"],
                         wnames["w33"], wnames["b33"], v_ds[2], out_d=out_d)
    nc.finalize()
    return nc


def pack_wblob(inputs):
    lay, tot = _blob_layout()
    blob = np.empty(tot, np.float32)
    for li in range(3):
        for nm, (off, shp) in lay[li].items():
            a = np.asarray(inputs[nm], dtype=np.float32).reshape(-1)
            blob[off:off + a.size] = a
    return blob


def make_in_maps(inputs):
    pos = np.ascontiguousarray(np.asarray(inputs["pos"], dtype=np.float32))
    blob = pack_wblob(inputs)
    return [{"pos": pos[b], "wblob": blob} for b in range(B)]


def _make_runner(nc):
    """Cached jitted NEFF executor: warm kernel() calls skip retracing.
    The weight blob is passed replicated (one host copy, not an 8x concat)."""
    import jax
    from jax.sharding import Mesh, PartitionSpec
    from jax.experimental.shard_map import shard_map
    from concourse.bass2jax import (_bass_exec_p, install_neuronx_cc_hook,
                                    partition_id_tensor)
    install_neuronx_cc_hook()
    partition_name = nc.partition_id_tensor.name if nc.partition_id_tensor else None
    in_names, out_names, out_avals, zero_shapes = [], [], [], []
    for alloc in nc.m.functions[0].allocations:
        if not isinstance(alloc, mybir.MemoryLocationSet):
            continue
        name = alloc.memorylocations[0].name
        if alloc.kind == "ExternalInput":
            if name != partition_name:
                in_names.append(name)
        elif alloc.kind == "ExternalOutput":
            out_names.append(name)
            shape = tuple(alloc.tensor_shape)
            dtype = mybir.dt.np(alloc.dtype)
            out_avals.append(jax.core.ShapedArray(shape, dtype))
            zero_shapes.append((shape, dtype))
    n_params = len(in_names)
    n_outs = len(out_avals)
    in_names_all = list(in_names) + out_names
    if partition_name is not None:
        in_names_all.append(partition_name)

    def _body(*args):
        operands = list(args)
        if partition_name is not None:
            operands.append(partition_id_tensor())
        return tuple(_bass_exec_p.bind(
            *operands, out_avals=tuple(out_avals),
            in_names=tuple(in_names_all), out_names=tuple(out_names),
            lowering_input_output_aliases=(),
            sim_require_finite=True, sim_require_nnan=True, nc=nc))

    devices = jax.devices()[:B]
    assert len(devices) >= B or len(devices) == B
    mesh = Mesh(np.asarray(devices), ("core",))
    rep_names = {"wblob"} & set(in_names)
    in_specs = tuple(
        PartitionSpec() if nm in rep_names else PartitionSpec("core")
        for nm in in_names
    ) + (PartitionSpec("core"),) * n_outs
    sharded = jax.jit(
        shard_map(_body, mesh=mesh, in_specs=in_specs,
                  out_specs=(PartitionSpec("core"),) * len(out_names),
                  check_rep=False),
        donate_argnums=tuple(range(n_params, n_params + n_outs)),
        keep_unused=True,
    )

    # Output buffers are pure scratch (the kernel fully overwrites out_d), so
    # they are chained across calls via donation: the first call ships zeros,
    # every later call re-donates the previous device-resident buffers. This
    # removes an 8 MB host->device upload (~80 ms of axon RPC) per warm call.
    state = {"outs": None}

    def submit(in_maps):
        """Upload fresh inputs, launch, return device output handles."""
        per_core = [[np.asarray(m[name]) for name in in_names] for m in in_maps]
        args_in = [
            per_core[0][i] if in_names[i] in rep_names else
            np.concatenate([per_core[c][i] for c in range(B)], axis=0)
            for i in range(n_params)
        ]
        outs = state["outs"]
        if outs is None:
            outs = [np.zeros((B * sh[0], *sh[1:]), dt) for sh, dt in zero_shapes]
        out_arrs = list(sharded(*args_in, *outs))
        state["outs"] = out_arrs
        jax.block_until_ready(out_arrs)
        return out_arrs

    def run(in_maps):
        out_arrs = submit(in_maps)
        return [
            {name: np.asarray(out_arrs[i]).reshape(B, *out_avals[i].shape)[c]
             for i, name in enumerate(out_names)}
            for c in range(B)
        ]

    run.submit = submit
    return run


def kernel(**inputs):
    if "nc" not in _CACHE:
        _CACHE["nc"] = build()
        _CACHE["run"] = _make_runner(_CACHE["nc"])
    in_maps = make_in_maps(inputs)
    results = _CACHE["run"](in_maps)
    out = np.stack([results[b]["out"] for b in range(B)], axis=0)
    return out


if __name__ == "__main__":
    rng = np.random.default_rng(0)
    fake = {"pos": rng.standard_normal((B, N, 3)).astype(np.float32)}
    for pfx in ("1", "2", "3"):
        d2 = 6 if pfx == "1" else 128
        fake[f"w{pfx}1"] = rng.standard_normal((d2, H)).astype(np.float32) * 0.2
        fake[f"w{pfx}2"] = rng.standard_normal((H, H)).astype(np.float32) * 0.12
        fake[f"w{pfx}3"] = rng.standard_normal((H, H)).astype(np.float32) * 0.12
        for j in ("1", "2", "3"):
            fake[f"b{pfx}{j}"] = np.zeros(H, np.float32)
    o = kernel(**fake)
    print("out", o.shape, o.dtype, float(np.abs(o).max()))

